# revision 10
# baseline (speedup 1.0000x reference)
"""Trainium2 Bass kernel for the S4-reservoir layer (nn_S4R_58308476010695).

Math: y = tanh(causal_conv(u, K) + D*u);  out = GLU(W_mix @ y + b_mix)
where K[h,l] = 2*Re(sum_n CB[h,n] * Lambda[h,n]^l).

|Lambda| <= 0.99 so K decays geometrically; the banded block-Toeplitz
truncation covers lags [0, (DLAG-1)*128 + r] for output position r
within a block (DLAG=4 -> rel err ~3.3e-3, gate 2e-2; DLAG=3 -> 1.4e-2).

Phase 1 (conv, H-sharded 32ch/core) computes the banded block-Toeplitz
conv with the TOEPLITZ tile as the stationary operand:

  y[h][r, (j,b)] = sum_d  W1[h,d][s, r]^T @ u_window_d[s, (j,b)]

One LoadStationary per (h,d) feeds a single 256-column moving pass (all
32 j-blocks x 8 batch), so the PE is MM-rate-bound (~107ns/tile) instead
of LS-bound.  Output lands transposed [r, jb]; 2 channels share one PSUM
bank and one [128,512] tanh.

Reshard: 2 x 1MB fp16 AllToAlls (chunk = 16 channels).  A tiny warmup
AllToAll with no input deps is instruction one - the ncfw pickup+prep
(~11-15us, one-time) runs during phase 1 instead of gating mesh-0.
Scatter DMAs are spread over 4 engine queues; payload layout is
receiver-contiguous (8KB per channel), so rhs assembly is 4 big DMAs.

Phase 2 (1x1 mix + GLU, L-sharded 512 pos/core): contraction chunk c ==
AllToAll chunk c; c0 matmuls overlap AllToAll-1; per kcol GLU = ACT
sigmoid + DVE scalar_tensor_tensor, out DMA per (kcol, og).
"""

import numpy as np

import concourse.bass as bass
import concourse.mybir as mybir
import concourse.tile as tile
from concourse import bacc, bass_utils

B, H, L, N = 8, 256, 4096, 64
T = 128            # conv block size = matmul contraction dim
DLAG = 4           # kernel truncation: effective lags [0, 384+r] per out pos r
NCORE = 8
HL = H // NCORE    # 32 channels per core in phase 1
J = L // T         # 32 blocks per sequence
LS = L // NCORE    # 512 positions per core in phase 2
JJ = LS // T       # 4 j-blocks per core's L-slice
HC = HL // 2       # 16 channels per AllToAll chunk
PAD = (DLAG - 1) * B          # zero cols so shifted u windows stay in-channel
CPW = PAD + J * B             # u cols per channel
JB = J * B                    # 256 (j,b) columns per channel

F16 = mybir.dt.float16
F32 = mybir.dt.float32
AF = mybir.ActivationFunctionType

# test.py pokes these for profiling
last_results = None
run_kwargs = {}


def _build_program():
    nc = bacc.Bacc(num_devices=NCORE)
    u_d = nc.declare_dram_parameter("u_arr", [T, HL * CPW], F16, False)
    w1_d = nc.declare_dram_parameter("w1", [T, HL * DLAG * T], F16, False)
    w2_d = nc.declare_dram_parameter("w2", [T, 2 * 512], F16, False)
    bb_d = nc.declare_dram_parameter("bb", [T, 4], F32, False)
    # out rows = GLU channel (o), cols = (r, jj, b); host un-permutes (free).
    out_d = nc.declare_dram_parameter("out", [2 * T, B * LS], F16, True)

    DT = DLAG * T
    with tile.TileContext(nc) as tc:
        with tc.tile_pool(name="const", bufs=1) as cpool, \
             tc.tile_pool(name="dram", bufs=1, space="DRAM") as dpool:
            # ---- warmup collective: no input deps -> doorbell at t~0;
            # ncfw boot + first-collective prep hide under phase 1.
            wu_in = dpool.tile([NCORE, 64], F16, tag="wui", name="wu_in")
            wu_out = dpool.tile([NCORE, 64], F16, tag="wuo", name="wu_out")
            nc.gpsimd.collective_compute(
                "AllToAll", mybir.AluOpType.bypass,
                replica_groups=[list(range(NCORE))],
                ins=[wu_in.opt()], outs=[wu_out.opt()],
            )

            # ---- input loads in consumption order on two queues
            u_sb = cpool.tile([T, HL * CPW], F16, tag="u", name="u_sb")
            w1_sb = cpool.tile([T, HL * DT], F16, tag="w1", name="w1_sb")
            nc.sync.dma_start(w1_sb[:, :2 * DT], w1_d[:, :2 * DT])          # ch 0-1
            nc.scalar.dma_start(u_sb[:, :4 * CPW], u_d[:, :4 * CPW])        # ch 0-3
            nc.sync.dma_start(w1_sb[:, 2 * DT:8 * DT], w1_d[:, 2 * DT:8 * DT])
            nc.scalar.dma_start(u_sb[:, 4 * CPW:16 * CPW], u_d[:, 4 * CPW:16 * CPW])
            nc.sync.dma_start(w1_sb[:, 8 * DT:20 * DT], w1_d[:, 8 * DT:20 * DT])
            nc.scalar.dma_start(u_sb[:, 16 * CPW:], u_d[:, 16 * CPW:])
            nc.sync.dma_start(w1_sb[:, 20 * DT:], w1_d[:, 20 * DT:])
            w2_sb = cpool.tile([T, 2 * 512], F16, tag="w2", name="w2_sb")
            nc.gpsimd.dma_start(w2_sb[:], w2_d[:])
            bb_sb = cpool.tile([T, 4], F32, tag="bb", name="bb_sb")
            nc.gpsimd.dma_start(bb_sb[:], bb_d[:])

            spool = tc.alloc_tile_pool(name="sig", bufs=4)
            # preload the tanh ACT table during the load window
            tw = spool.tile([T, 1], F16, name="tanhw", tag="sigw")
            nc.scalar.activation(tw[:], bb_sb[:, 0:1], AF.Tanh)

            # collective buffers, chunk k: [dest][h][r][jb]; after AllToAll
            # the receiver reads [src][h] -> one 8KB contiguous (r, jb) run
            # per global channel = one rhs partition of phase 2.
            cc_in = [dpool.tile([NCORE, HC, T, 32], F16, tag=f"cci{k}", name=f"cc_in{k}")
                     for k in range(2)]
            cc_out = [dpool.tile([NCORE, HC, T, 32], F16, tag=f"cco{k}", name=f"cc_out{k}")
                      for k in range(2)]

            # tanh staging: per chunk a [r, (h, jb)] tile
            stag = [cpool.tile([T, HC * JB], F16, tag=f"st{k}", name=f"stag{k}")
                    for k in range(2)]

            rpool = tc.alloc_tile_pool(name="rhs", bufs=1)
            opool = tc.alloc_tile_pool(name="ostg", bufs=1)

            dma_engs = [nc.sync, nc.scalar, nc.gpsimd]

            # ---- phase 1: transposed banded Toeplitz conv + tanh ----
            with tc.tile_pool(name="psum1", bufs=8, space="PSUM") as pp1:
                for k in range(2):           # AllToAll chunk (16 channels)
                    for hh in range(0, HC, 2):   # channel pair -> one bank
                        ps = pp1.tile([T, 2 * JB], F32, name=f"ps{k}_{hh}", tag="ps")
                        for t_h in range(2):
                            h = k * HC + hh + t_h
                            for d in range(DLAG):
                                c0 = h * CPW + PAD - d * B
                                nc.tensor.matmul(
                                    ps[:, t_h * JB:(t_h + 1) * JB],
                                    lhsT=w1_sb[:, (h * DLAG + d) * T:(h * DLAG + d + 1) * T],
                                    rhs=u_sb[:, c0:c0 + JB],
                                    start=(d == 0),
                                    stop=(d == DLAG - 1),
                                )
                        nc.scalar.activation(
                            stag[k][:, hh * JB:(hh + 2) * JB], ps[:], AF.Tanh)
                    # scatter chunk k: 8 dmas (one per dest core), each
                    # [128 r, 16 h, 32 jb] - 64B runs, spread over 4 queues.
                    src_v = stag[k].rearrange("p (h j) -> p h j", h=HC)
                    for dq in range(NCORE):
                        nc_eng = dma_engs[dq % 3]
                        nc_eng.dma_start(
                            cc_in[k][dq].rearrange("h r j -> r h j"),
                            src_v[:, :, 32 * dq:32 * dq + 32])
                    nc.gpsimd.collective_compute(
                        "AllToAll", mybir.AluOpType.bypass,
                        replica_groups=[list(range(NCORE))],
                        ins=[cc_in[k].opt()], outs=[cc_out[k].opt()],
                    )

            # preload the sigmoid activation table while ACT is idle
            sigw = spool.tile([T, 1], F16, name="sigw", tag="sigw")
            nc.scalar.activation(sigw[:], bb_sb[:, 0:1], AF.Sigmoid)

            # ---- rhs assembly: 4 contiguous DMAs per chunk on 2 queues
            rhs_ts = []
            for c in range(2):
                rt = rpool.tile([T, 32 * T], F16, tag=f"rhs{c}", name=f"rhs{c}")
                src_v = cc_out[c].rearrange("s h r j -> (s h) (r j)")
                for q in range(4):
                    eng = (nc.sync, nc.scalar)[q % 2]
                    eng.dma_start(rt[32 * q:32 * q + 32, :],
                                  src_v[32 * q:32 * q + 32])
                rhs_ts.append(rt)

            # ---- phase 2: 1x1 channel mix + GLU on this core's L slice ----
            # contraction chunk c == AllToAll chunk c (w2 rows permuted on
            # host), c0 emitted two kcols ahead (PSUM-capacity limit) so
            # chunk-0 matmuls overlap the second AllToAll; out per kcol.
            OS = [opool.tile([T, B * LS], F16, tag=f"os{og}", name=f"os{og}")
                  for og in range(2)]
            NK = B * LS // 512          # 8 col chunks of 512
            with tc.tile_pool(name="psum2", bufs=8, space="PSUM") as pp2:
                pss = {}

                def emit_mm(kcol, c):
                    if c == 0:
                        pss[kcol] = [pp2.tile([T, 512], F32, name=f"z{kcol}_{ot}", tag="z")
                                     for ot in range(4)]
                    for ot in range(4):
                        nc.tensor.matmul(
                            pss[kcol][ot][:],
                            lhsT=w2_sb[:, c * 512 + ot * T:c * 512 + (ot + 1) * T],
                            rhs=rhs_ts[c][:, kcol * 512:(kcol + 1) * 512],
                            start=(c == 0),
                            stop=(c == 1),
                        )

                def emit_glu(kcol):
                    for og in range(2):
                        sig = spool.tile([T, 512], F16, name=f"sig{kcol}_{og}", tag="sig")
                        nc.scalar.activation(sig[:], pss[kcol][og + 2][:], AF.Sigmoid,
                                             bias=bb_sb[:, og + 2:og + 3])
                        nc.vector.scalar_tensor_tensor(
                            OS[og][:, kcol * 512:(kcol + 1) * 512],
                            pss[kcol][og][:], bb_sb[:, og:og + 1], sig[:],
                            op0=mybir.AluOpType.add, op1=mybir.AluOpType.mult)

                for i in range(NK + 2):
                    if i < NK:
                        emit_mm(i, 0)
                    if i >= 2:
                        kcol = i - 2
                        emit_mm(kcol, 1)
                        emit_glu(kcol)
                        for og in range(2):
                            dma_engs[(2 * kcol + og) % 3].dma_start(
                                out_d[og * T:(og + 1) * T,
                                      kcol * 512:(kcol + 1) * 512],
                                OS[og][:, kcol * 512:(kcol + 1) * 512])
            opool.release()
            rpool.release()
            spool.release()
    return nc


def _host_prep(u, Lambda_re, Lambda_im, CB_re, CB_im, D, W_mix, b_mix):
    Lam = Lambda_re.astype(np.complex128) + 1j * Lambda_im.astype(np.complex128)
    CB = CB_re.astype(np.complex128) + 1j * CB_im.astype(np.complex128)
    Lk = DLAG * T
    K = np.empty((H, Lk), np.float64)
    P = np.ones((H, N), np.complex128)
    for l in range(Lk):
        K[:, l] = 2.0 * (CB.real * P.real - CB.imag * P.imag).sum(axis=1)
        P *= Lam
    K[:, 0] += D.astype(np.float64)          # fold the skip connection into lag 0

    # stationary Toeplitz tiles: W1[h,d,s,r] = K[h, d*T + r - s] (0 out of band)
    Kbp = np.concatenate([np.zeros((H, T - 1)), K], axis=1)
    base = np.arange(T)[None, :] - np.arange(T)[:, None] + (T - 1)       # [s, r]
    idx = base[None, :, :] + (np.arange(DLAG) * T)[:, None, None]        # [d, s, r]
    W1 = Kbp[:, idx].astype(np.float16)                                  # [H, d, s, r]

    # w2 lhsT: partitions = channels of AllToAll chunk c in (src, h) order
    # col block (c, ot) holds W_mix rows ot*128..+128 for those channels.
    w2_arr = np.empty((T, 2 * 512), np.float16)
    for c in range(2):
        ch = (np.arange(T) // HC) * HL + c * HC + np.arange(T) % HC      # [128]
        for ot in range(4):
            w2_arr[:, c * 512 + ot * T:c * 512 + (ot + 1) * T] = \
                W_mix[ot * T:(ot + 1) * T, ch].T
    bb_arr = np.ascontiguousarray(b_mix.reshape(4, T).T).astype(np.float32)

    in_maps = []
    for c in range(NCORE):
        h0 = c * HL
        x = u[:, h0:h0 + HL].reshape(B, HL, J, T).transpose(3, 1, 2, 0)  # [s,h,j,b]
        x = x.reshape(T, HL, J * B)
        u_arr = np.concatenate(
            [np.zeros((T, HL, PAD), np.float32), x], axis=2
        ).reshape(T, HL * CPW).astype(np.float16)
        w1_arr = np.ascontiguousarray(
            W1[h0:h0 + HL].transpose(2, 0, 1, 3)).reshape(T, HL * DLAG * T)
        in_maps.append({"u_arr": u_arr, "w1": w1_arr, "w2": w2_arr, "bb": bb_arr})
    return in_maps


def kernel(u, Lambda_re, Lambda_im, CB_re, CB_im, D, W_mix, b_mix):
    global last_results
    args = [np.asarray(x) for x in
            (u, Lambda_re, Lambda_im, CB_re, CB_im, D, W_mix, b_mix)]
    in_maps = _host_prep(*args)
    nc = _build_program()
    nc.compile()   # bacc passes: wait legalization, regalloc, DCE
    res = bass_utils.run_bass_kernel_spmd(nc, in_maps, list(range(NCORE)), **run_kwargs)
    last_results = res
    out = np.empty((B, H, L), np.float32)
    for c in range(NCORE):
        # device rows = GLU channel o, cols = (r, jj, b); core holds
        # l in [c*LS, (c+1)*LS), l_local = jj*T + r.
        a = res.results[c]["out"].astype(np.float32).reshape(H, T, JJ, B)
        out[:, :, c * LS:(c + 1) * LS] = (
            a.transpose(3, 0, 2, 1).reshape(B, H, LS))
    return out


# revision 13
# speedup vs baseline: 1.0696x; 1.0696x over previous
"""Trainium2 Bass kernel for the S4-reservoir layer (nn_S4R_58308476010695).

Math: y = tanh(causal_conv(u, K) + D*u);  out = GLU(W_mix @ y + b_mix)
where K[h,l] = 2*Re(sum_n CB[h,n] * Lambda[h,n]^l).

|Lambda| <= 0.99 so K decays geometrically; the banded block-Toeplitz
truncation covers lags [0, (DLAG-1)*128 + r] for output position r
within a block (DLAG=4 -> rel err ~3.3e-3, gate 2e-2; DLAG=3 -> 1.4e-2).

The conv is a banded block-Toeplitz matmul computed TRANSPOSED: for each
channel h and output half g (j-blocks 16g..16g+15):

  yT[(j,b), r] = sum_d  u_window_d[s, (j,b)]^T @ W1[h,d][s, r]

with lhsT (stationary) = a shifted 128-column window of u (LoadStationary
is hidden by the PE's reorder window at N=128) and rhs (moving) = the
Toeplitz tile W1[h,d][s,r] = K[h, d*T + r - s].  Producing y transposed
(positions on partitions) keeps every reshard DMA at 256B contiguous runs
- small-run DMA descriptor floods on the SDMA engines delay the
collectives' mesh start, so run size is critical.

A PSUM bank holds 4 accumulation groups (2 channels x 2 halves), so tanh
runs as 16 x [128,512] ACT ops (~720ns each) instead of 64 small ones.

Sharding: phase 1 (conv + tanh) over H (32 ch/core); phase 2 (1x1 mix +
GLU, contracts all 256 channels) over L (512 positions/core).  Reshard =
two 1MB fp16 AllToAlls; the first mesh starts ~11us after the LAST
chunk's scatter completes (ncfw prep), so the scatter is split over all
3 DMA-capable queues to finish right behind the tanhs.  Phase-2
contraction chunk c == AllToAll chunk c; chunk-0 matmuls overlap
AllToAll-1, GLU = ACT sigmoid + DVE scalar_tensor_tensor per kcol.
"""

import numpy as np

import concourse.bass as bass
import concourse.mybir as mybir
import concourse.tile as tile
from concourse import bacc, bass_utils

B, H, L, N = 8, 256, 4096, 64
T = 128            # conv block size = matmul contraction dim
DLAG = 4           # kernel truncation: effective lags [0, 384+r] per out pos r
NCORE = 8
HL = H // NCORE    # 32 channels per core in phase 1
J = L // T         # 32 blocks per sequence
LS = L // NCORE    # 512 positions per core in phase 2
JJ = LS // T       # 4 j-blocks per core's L-slice
HC = HL // 2       # 16 channels per AllToAll chunk
PAD = (DLAG - 1) * B          # zero cols so shifted u windows stay in-channel
CPW = PAD + J * B             # u cols per channel

F16 = mybir.dt.float16
F32 = mybir.dt.float32
AF = mybir.ActivationFunctionType

# test.py pokes these for profiling
last_results = None
run_kwargs = {}


def _build_program():
    nc = bacc.Bacc(num_devices=NCORE)
    u_d = nc.declare_dram_parameter("u_arr", [T, HL * CPW], F16, False)
    w1_d = nc.declare_dram_parameter("w1", [T, HL * DLAG * T], F16, False)
    w2_d = nc.declare_dram_parameter("w2", [T, 2 * 512], F16, False)
    bb_d = nc.declare_dram_parameter("bb", [T, 4], F32, False)
    # out rows = GLU channel (o), cols = (jj, b, r); host un-permutes (free).
    out_d = nc.declare_dram_parameter("out", [2 * T, B * LS], F16, True)

    DT = DLAG * T
    with tile.TileContext(nc) as tc:
        with tc.tile_pool(name="const", bufs=1) as cpool, \
             tc.tile_pool(name="dram", bufs=1, space="DRAM") as dpool:
            # ---- input loads: first channels' tiles first so PE starts early
            bb_sb = cpool.tile([T, 4], F32, tag="bb", name="bb_sb")
            nc.sync.dma_start(bb_sb[:], bb_d[:])
            u_sb = cpool.tile([T, HL * CPW], F16, tag="u", name="u_sb")
            w1_sb = cpool.tile([T, HL * DT], F16, tag="w1", name="w1_sb")
            nc.sync.dma_start(w1_sb[:, :2 * DT], w1_d[:, :2 * DT])          # ch 0-1
            nc.sync.dma_start(u_sb[:, :4 * CPW], u_d[:, :4 * CPW])          # ch 0-3
            nc.sync.dma_start(w1_sb[:, 2 * DT:8 * DT], w1_d[:, 2 * DT:8 * DT])
            nc.sync.dma_start(u_sb[:, 4 * CPW:16 * CPW], u_d[:, 4 * CPW:16 * CPW])
            nc.sync.dma_start(w1_sb[:, 8 * DT:20 * DT], w1_d[:, 8 * DT:20 * DT])
            nc.sync.dma_start(u_sb[:, 16 * CPW:], u_d[:, 16 * CPW:])
            nc.sync.dma_start(w1_sb[:, 20 * DT:], w1_d[:, 20 * DT:])
            w2_sb = cpool.tile([T, 2 * 512], F16, tag="w2", name="w2_sb")
            nc.gpsimd.dma_start(w2_sb[:], w2_d[:])

            spool = tc.alloc_tile_pool(name="sig", bufs=4)
            # preload the tanh ACT table during the load window
            tw = spool.tile([T, 1], F16, name="tanhw", tag="sigw")
            nc.scalar.activation(tw[:], bb_sb[:, 0:1], AF.Tanh)

            # collective buffers, chunk k: [dest][h][p32][r]; after AllToAll
            # the receiver reads [src][h] -> one 8KB contiguous (p32, r) run
            # per global channel = one rhs partition of phase 2.  (A merged
            # 2MB AllToAll NaNs on this stack - keep two 1MB ones.)
            cc_in = [dpool.tile([NCORE, HC, 32, T], F16, tag=f"cci{k}", name=f"cc_in{k}")
                     for k in range(2)]
            cc_out = [dpool.tile([NCORE, HC, 32, T], F16, tag=f"cco{k}", name=f"cc_out{k}")
                      for k in range(2)]

            # tanh staging: per chunk [jb, (h-pair, g, t_h, r)] matching the
            # PSUM bank column order (g, t_h) so one tanh covers 4 groups.
            stag = [cpool.tile([T, HC * 512 // 2], F16, tag=f"st{k}", name=f"stag{k}")
                    for k in range(2)]

            rpool = tc.alloc_tile_pool(name="rhs", bufs=1)
            opool = tc.alloc_tile_pool(name="ostg", bufs=1)

            dma_engs = [nc.sync, nc.scalar, nc.gpsimd]

            # ---- phase 1: transposed banded Toeplitz conv + tanh ----
            with tc.tile_pool(name="psum1", bufs=8, space="PSUM") as pp1:
                for k in range(2):           # AllToAll chunk (16 channels)
                    for hh in range(0, HC, 2):   # channel pair -> one bank
                        ps = pp1.tile([T, 512], F32, name=f"ps{k}_{hh}", tag="ps")
                        for g in range(2):       # output half: j in [16g, 16g+16)
                            for t_h in range(2):
                                h = k * HC + hh + t_h
                                q = t_h * 2 + g   # bank cols = (t_h, g, r)
                                for d in range(DLAG):
                                    c0 = h * CPW + PAD + (16 * g - d) * B
                                    nc.tensor.matmul(
                                        ps[:, q * T:(q + 1) * T],
                                        lhsT=u_sb[:, c0:c0 + T],
                                        rhs=w1_sb[:, (h * DLAG + d) * T:(h * DLAG + d + 1) * T],
                                        start=(d == 0),
                                        stop=(d == DLAG - 1),
                                    )
                        nc.scalar.activation(
                            stag[k][:, hh * 256:(hh + 2) * 256], ps[:], AF.Tanh)
                    # scatter chunk k: 8 dmas (2 g x 4 dest) on 3 queues, each
                    # [32 jb, 16 h, 128 r] 3-dim APs - 256B runs.
                    src_v = stag[k].rearrange("p (h g r) -> p h g r", h=HC, g=2)
                    for g in range(2):
                        for dq in range(4):
                            dst_v = cc_in[k][4 * g + dq].rearrange("h p r -> p h r")
                            dma_engs[(4 * g + dq) % 3].dma_start(
                                dst_v, src_v[32 * dq:32 * dq + 32, :, g])
                    nc.gpsimd.collective_compute(
                        "AllToAll", mybir.AluOpType.bypass,
                        replica_groups=[list(range(NCORE))],
                        ins=[cc_in[k].opt()], outs=[cc_out[k].opt()],
                    )

            # preload the sigmoid activation table while ACT is idle
            sigw = spool.tile([T, 1], F16, name="sigw", tag="sigw")
            nc.scalar.activation(sigw[:], bb_sb[:, 0:1], AF.Sigmoid)

            # ---- rhs assembly: 4 contiguous DMAs per chunk on 2 queues
            rhs_ts = []
            for c in range(2):
                rt = rpool.tile([T, 32 * T], F16, tag=f"rhs{c}", name=f"rhs{c}")
                src_v = cc_out[c].rearrange("s h p r -> (s h) (p r)")
                for q in range(4):
                    eng = (nc.sync, nc.scalar)[q % 2]
                    eng.dma_start(rt[32 * q:32 * q + 32, :],
                                  src_v[32 * q:32 * q + 32])
                rhs_ts.append(rt)

            # ---- phase 2: 1x1 channel mix + GLU on this core's L slice ----
            # contraction chunk c == AllToAll chunk c (w2 rows permuted on
            # host), c0 emitted two kcols ahead (PSUM-capacity limit) so
            # chunk-0 matmuls overlap the second AllToAll; out per kcol.
            OS = [opool.tile([T, B * LS], F16, tag=f"os{og}", name=f"os{og}")
                  for og in range(2)]
            NK = B * LS // 512          # 8 col chunks of 512
            with tc.tile_pool(name="psum2", bufs=8, space="PSUM") as pp2:
                pss = {}

                def emit_mm(kcol, c):
                    if c == 0:
                        pss[kcol] = [pp2.tile([T, 512], F32, name=f"z{kcol}_{ot}", tag="z")
                                     for ot in range(4)]
                    for ot in range(4):
                        nc.tensor.matmul(
                            pss[kcol][ot][:],
                            lhsT=w2_sb[:, c * 512 + ot * T:c * 512 + (ot + 1) * T],
                            rhs=rhs_ts[c][:, kcol * 512:(kcol + 1) * 512],
                            start=(c == 0),
                            stop=(c == 1),
                        )

                def emit_glu(kcol):
                    for og in range(2):
                        sig = spool.tile([T, 512], F16, name=f"sig{kcol}_{og}", tag="sig")
                        nc.scalar.activation(sig[:], pss[kcol][og + 2][:], AF.Sigmoid,
                                             bias=bb_sb[:, og + 2:og + 3])
                        nc.vector.scalar_tensor_tensor(
                            OS[og][:, kcol * 512:(kcol + 1) * 512],
                            pss[kcol][og][:], bb_sb[:, og:og + 1], sig[:],
                            op0=mybir.AluOpType.add, op1=mybir.AluOpType.mult)

                for i in range(NK + 2):
                    if i < NK:
                        emit_mm(i, 0)
                    if i >= 2:
                        kcol = i - 2
                        emit_mm(kcol, 1)
                        emit_glu(kcol)
                        for og in range(2):
                            dma_engs[(2 * kcol + og) % 3].dma_start(
                                out_d[og * T:(og + 1) * T,
                                      kcol * 512:(kcol + 1) * 512],
                                OS[og][:, kcol * 512:(kcol + 1) * 512])
            opool.release()
            rpool.release()
            spool.release()
    return nc


def _host_prep(u, Lambda_re, Lambda_im, CB_re, CB_im, D, W_mix, b_mix):
    Lam = Lambda_re.astype(np.complex128) + 1j * Lambda_im.astype(np.complex128)
    CB = CB_re.astype(np.complex128) + 1j * CB_im.astype(np.complex128)
    Lk = DLAG * T
    K = np.empty((H, Lk), np.float64)
    P = np.ones((H, N), np.complex128)
    for l in range(Lk):
        K[:, l] = 2.0 * (CB.real * P.real - CB.imag * P.imag).sum(axis=1)
        P *= Lam
    K[:, 0] += D.astype(np.float64)          # fold the skip connection into lag 0

    # rhs Toeplitz tiles: W1[h,d,s,r] = K[h, d*T + r - s] (0 when out of band)
    Kbp = np.concatenate([np.zeros((H, T - 1)), K], axis=1)
    base = np.arange(T)[None, :] - np.arange(T)[:, None] + (T - 1)       # [s, r]
    idx = base[None, :, :] + (np.arange(DLAG) * T)[:, None, None]        # [d, s, r]
    W1 = Kbp[:, idx].astype(np.float16)                                  # [H, d, s, r]

    # w2 lhsT: partitions = channels of AllToAll chunk c in (src, h) order
    # col block (c, ot) holds W_mix rows ot*128..+128 for those channels.
    w2_arr = np.empty((T, 2 * 512), np.float16)
    for c in range(2):
        ch = (np.arange(T) // HC) * HL + c * HC + np.arange(T) % HC      # [128]
        for ot in range(4):
            w2_arr[:, c * 512 + ot * T:c * 512 + (ot + 1) * T] = \
                W_mix[ot * T:(ot + 1) * T, ch].T
    bb_arr = np.ascontiguousarray(b_mix.reshape(4, T).T).astype(np.float32)

    in_maps = []
    for c in range(NCORE):
        h0 = c * HL
        x = u[:, h0:h0 + HL].reshape(B, HL, J, T).transpose(3, 1, 2, 0)  # [s,h,j,b]
        x = x.reshape(T, HL, J * B)
        u_arr = np.concatenate(
            [np.zeros((T, HL, PAD), np.float32), x], axis=2
        ).reshape(T, HL * CPW).astype(np.float16)
        w1_arr = np.ascontiguousarray(
            W1[h0:h0 + HL].transpose(2, 0, 1, 3)).reshape(T, HL * DLAG * T)
        in_maps.append({"u_arr": u_arr, "w1": w1_arr, "w2": w2_arr, "bb": bb_arr})
    return in_maps


def kernel(u, Lambda_re, Lambda_im, CB_re, CB_im, D, W_mix, b_mix):
    global last_results
    args = [np.asarray(x) for x in
            (u, Lambda_re, Lambda_im, CB_re, CB_im, D, W_mix, b_mix)]
    in_maps = _host_prep(*args)
    nc = _build_program()
    nc.compile()   # bacc passes: wait legalization, regalloc, DCE
    res = bass_utils.run_bass_kernel_spmd(nc, in_maps, list(range(NCORE)), **run_kwargs)
    last_results = res
    out = np.empty((B, H, L), np.float32)
    for c in range(NCORE):
        # device rows = GLU channel o, cols = (jj, b, r); core holds
        # l in [c*LS, (c+1)*LS), l_local = jj*T + r.
        a = res.results[c]["out"].astype(np.float32).reshape(H, JJ, B, T)
        out[:, :, c * LS:(c + 1) * LS] = (
            a.transpose(2, 0, 1, 3).reshape(B, H, LS))
    return out
